# revision 9
# baseline (speedup 1.0000x reference)
"""Trainium2 Bass kernel for nn_Attention_68504728371431.

Reference computation:
  theta_x = theta_w @ x + theta_b    [B, Ci, N] (1x1 conv)
  phi_x   = phi_w @ x + phi_b
  g_x     = g_w @ x + g_b
  f  = theta_x^T phi_x / N           [B, N, N]  (no softmax!)
  y  = f @ g_x^T                     [B, N, Ci]
  wy = w_w @ y^T + w_b               [B, C, N]
  out = BN(wy) * gamma + beta + x    (BN over B,H,W per channel)

Algebraic restructuring (f is linear, so associativity applies):
  y^T = P^T @ theta_x'  with  P = sum_m phi_x[:,m] g_x[m,:]   [Ci, Ci]
  theta' scaled by 1/N on the host. The N x N attention matrix never
  exists; total work drops ~64x. Also, w_b cancels under BN (per-channel
  shift), so it is dropped entirely.

Sharding: 8 cores = 4 batches x 2 column-halves of N. Host permutes each
core's x so its own half is always cols [0:2048) (P is invariant to column
order). Each core computes P redundantly for its batch (cheap) - no
cross-core communication is needed until BN statistics.

Cross-batch BN stats: an on-device AllReduce measures ~250us on this
fabric - far more than the whole kernel - so the kernel runs as TWO NEFF
launches: (A) projections, P, yT, wy matmuls + per-core mean/var via
bn_stats/bn_aggr read directly from PSUM; host combines 8 cores' moments
(exact parallel-variance) into per-channel scale/shift; (B) recompute wy
from the yT handoff, apply the BN affine + residual, store.

Matmuls run in float32r (TF32-like, ~1.6e-4 rel err) except the P
accumulation which is plain fp32 (same speed at free-dim 128, exact).
Stats and the residual path are fp32.
"""

import numpy as np
from contextlib import ExitStack

import concourse.bass as bass
import concourse.tile as tile
from concourse import bacc, mybir
from concourse import bass2jax

B, C, CI, H, W = 4, 256, 128, 64, 64
N = H * W            # 4096
HALF = N // 2        # 2048
NCORES = 8
EPS = 1e-5
F32 = mybir.dt.float32
F32R = mybir.dt.float32r
AF = mybir.ActivationFunctionType

_CACHE = {}

NCHUNK = N // 128    # 32 m-chunks
NDBL = NCHUNK // 2   # 16 double-chunks
NT = HALF // 512     # 4 512-wide tiles per half
PLAG = 3             # P-matmul lag (double-chunks) behind the T-sweep


def _build_nc_a():
    nc = bacc.Bacc("TRN2", target_bir_lowering=False, debug=False,
                   num_devices=NCORES)

    x_in = nc.declare_dram_parameter("x", [2, 128, N], F32, isOutput=False)
    wcat_in = nc.declare_dram_parameter("wcat", [2, 128, 2 * CI], F32, isOutput=False)
    bcat_in = nc.declare_dram_parameter("bcat", [1, 2 * CI], F32, isOutput=False)
    thw_in = nc.declare_dram_parameter("thw", [2, 128, CI], F32, isOutput=False)
    thb_in = nc.declare_dram_parameter("thb", [CI, 1], F32, isOutput=False)
    wwt_in = nc.declare_dram_parameter("wwt", [CI, C], F32, isOutput=False)

    yt_out = nc.declare_dram_parameter("yt", [CI, HALF], F32R, isOutput=True)
    st_out = nc.declare_dram_parameter("st", [2, 128, 2], F32, isOutput=True)

    with tile.TileContext(nc) as tc, ExitStack() as ctx:
        const = ctx.enter_context(tc.tile_pool(name="const", bufs=1))
        xr = ctx.enter_context(tc.tile_pool(name="xr", bufs=1))
        work = ctx.enter_context(tc.tile_pool(name="work", bufs=2 * (PLAG + 2)))
        big = ctx.enter_context(tc.tile_pool(name="big", bufs=1))
        psA = ctx.enter_context(tc.tile_pool(name="psA", bufs=4, space="PSUM"))
        psP = ctx.enter_context(tc.tile_pool(name="psP", bufs=1, space="PSUM"))

        # ---- params ----
        wcat_f = [const.tile([128, 2 * CI], F32, name=f"wcatf{j}") for j in range(2)]
        wcat = [const.tile([128, 2 * CI], F32R, name=f"wcat{j}") for j in range(2)]
        thw_f = [const.tile([128, CI], F32, name=f"thwf{j}") for j in range(2)]
        thw = [const.tile([128, CI], F32R, name=f"thw{j}") for j in range(2)]
        wwt_f = const.tile([CI, C], F32)
        wwt = const.tile([CI, C], F32R)
        thb = const.tile([CI, 1], F32)
        bcat_f = const.tile([1, 2 * CI], F32)
        bcat_r = const.tile([1, 2 * CI], F32R)
        ones_f = const.tile([1, 128], F32)
        ones_r = const.tile([1, 128], F32R)
        for j in range(2):
            nc.sync.dma_start(wcat_f[j][:], wcat_in[j])
            nc.vector.tensor_copy(wcat[j][:], wcat_f[j][:])
            nc.sync.dma_start(thw_f[j][:], thw_in[j])
            nc.vector.tensor_copy(thw[j][:], thw_f[j][:])
        nc.sync.dma_start(wwt_f[:], wwt_in[:])
        nc.vector.tensor_copy(wwt[:], wwt_f[:])
        nc.sync.dma_start(thb[:], thb_in[:])
        nc.sync.dma_start(bcat_f[:], bcat_in[:])
        nc.vector.tensor_copy(bcat_r[:], bcat_f[:])
        nc.gpsimd.memset(ones_f[:], 1.0)
        nc.vector.tensor_copy(ones_r[:], ones_f[:])

        # ---- x: gpsimd casting DMA fp32 -> f32r, own half first ----
        x_r = [xr.tile([128, N], F32R, name=f"xr{j}") for j in range(2)]
        for k in range(4):
            for j in range(2):
                cs = slice(k * 1024, (k + 1) * 1024)
                nc.gpsimd.dma_start(x_r[j][:, cs], x_in[j, :, cs])

        # ---- T-sweep (transposed phi/g projections incl bias) + P ----
        # double-chunk d covers m-chunks 2d, 2d+1 in one PSUM bank
        p_ps = psP.tile([CI, CI], F32)
        tphg_tiles = []

        def emit_t(d):
            ps_t = psA.tile([128, 4 * CI], F32, tag="mm", name=f"ps_t{d}")
            for h in range(2):
                m = 2 * d + h
                ms = slice(m * 128, (m + 1) * 128)
                os_ = slice(h * 2 * CI, (h + 1) * 2 * CI)
                nc.tensor.matmul(ps_t[:, os_], ones_r[:], bcat_r[:],
                                 start=True, stop=False)
                nc.tensor.matmul(ps_t[:, os_], x_r[0][:, ms], wcat[0][:],
                                 start=False, stop=False)
                nc.tensor.matmul(ps_t[:, os_], x_r[1][:, ms], wcat[1][:],
                                 start=False, stop=True)
            tphg = work.tile([128, 4 * CI], F32, tag="tphg", name=f"tphg{d}")
            if d % 2 == 0:
                nc.vector.tensor_copy(tphg[:], ps_t[:])
            else:
                nc.scalar.copy(tphg[:], ps_t[:])
            tphg_tiles.append(tphg)

        def emit_p(d):
            t = tphg_tiles[d]
            for h in range(2):
                os_ = slice(h * 2 * CI, (h + 1) * 2 * CI)
                nc.tensor.matmul(p_ps[:], t[:, os_][:, 0:CI], t[:, os_][:, CI:2 * CI],
                                 start=(d == 0 and h == 0),
                                 stop=(d == NDBL - 1 and h == 1))

        for d in range(NDBL):
            emit_t(d)
            if d >= PLAG:
                emit_p(d - PLAG)
        for d in range(NDBL - PLAG, NDBL):
            emit_p(d)

        # ---- theta projection (own half, natural layout) ----
        ntheta = big.tile([CI, HALF], F32R)
        for t in range(NT):
            cs = slice(t * 512, (t + 1) * 512)
            ps_n = psA.tile([CI, 512], F32, tag="mm", name=f"ps_n{t}")
            nc.tensor.matmul(ps_n[:], thw[0][:], x_r[0][:, cs],
                             start=True, stop=False)
            nc.tensor.matmul(ps_n[:], thw[1][:], x_r[1][:, cs],
                             start=False, stop=True)
            nc.scalar.activation(ntheta[:, cs], ps_n[:], AF.Identity,
                                 bias=thb[:])

        p_sb = const.tile([CI, CI], F32R)
        nc.vector.tensor_copy(p_sb[:], p_ps[:])

        # ---- yT = P^T @ theta_x'; DMA out as soon as each tile is ready ----
        yt = big.tile([CI, HALF], F32R)
        for t in range(NT):
            cs = slice(t * 512, (t + 1) * 512)
            ps_y = psA.tile([CI, 512], F32, tag="mm", name=f"ps_y{t}")
            nc.tensor.matmul(ps_y[:], p_sb[:], ntheta[:, cs])
            nc.vector.tensor_copy(yt[:, cs], ps_y[:])
            nc.sync.dma_start(yt_out[:, cs], yt[:, cs])

        # ---- wy matmuls; bn_stats straight from PSUM (w_b dropped) ----
        st6 = [const.tile([128, 6 * NT], F32, name=f"st6_{j}") for j in range(2)]
        for t in range(NT):
            cs = slice(t * 512, (t + 1) * 512)
            for j in range(2):
                js = slice(j * 128, (j + 1) * 128)
                ps_w = psA.tile([128, 512], F32, tag="mm", name=f"ps_w{t}_{j}")
                nc.tensor.matmul(ps_w[:], wwt[:, js], yt[:, cs])
                nc.vector.bn_stats(st6[j][:, t * 6:(t + 1) * 6], ps_w[:])
        for j in range(2):
            stats = const.tile([128, 2], F32, name=f"stats{j}")
            nc.vector.bn_aggr(stats[:], st6[j][:])
            nc.sync.dma_start(st_out[j], stats[:])

    nc.compile()
    return nc


def _build_nc_b():
    nc = bacc.Bacc("TRN2", target_bir_lowering=False, debug=False,
                   num_devices=NCORES)

    yt_in = nc.declare_dram_parameter("yt", [CI, HALF], F32R, isOutput=False)
    xo_in = nc.declare_dram_parameter("xo", [2, 128, HALF], F32, isOutput=False)
    wwt_in = nc.declare_dram_parameter("wwt", [CI, C], F32, isOutput=False)
    sc_in = nc.declare_dram_parameter("sc", [2, 128, 1], F32, isOutput=False)
    sh_in = nc.declare_dram_parameter("sh", [2, 128, 1], F32, isOutput=False)
    out_d = nc.declare_dram_parameter("out", [2, 128, HALF], F32, isOutput=True)

    with tile.TileContext(nc) as tc, ExitStack() as ctx:
        const = ctx.enter_context(tc.tile_pool(name="const", bufs=1))
        xp = ctx.enter_context(tc.tile_pool(name="xp", bufs=4))
        work = ctx.enter_context(tc.tile_pool(name="work", bufs=4))
        psA = ctx.enter_context(tc.tile_pool(name="psA", bufs=4, space="PSUM"))

        wwt_f = const.tile([CI, C], F32)
        wwt = const.tile([CI, C], F32R)
        nc.sync.dma_start(wwt_f[:], wwt_in[:])
        nc.vector.tensor_copy(wwt[:], wwt_f[:])
        sc = [const.tile([128, 1], F32, name=f"sc{j}") for j in range(2)]
        sh = [const.tile([128, 1], F32, name=f"sh{j}") for j in range(2)]
        for j in range(2):
            nc.sync.dma_start(sc[j][:], sc_in[j])
            nc.sync.dma_start(sh[j][:], sh_in[j])
        yt = const.tile([CI, HALF], F32R)
        for t in range(NT):
            cs = slice(t * 512, (t + 1) * 512)
            nc.sync.dma_start(yt[:, cs], yt_in[:, cs])

        for t in range(NT):
            cs = slice(t * 512, (t + 1) * 512)
            for j in range(2):
                js = slice(j * 128, (j + 1) * 128)
                xt = xp.tile([128, 512], F32, tag="xt", name=f"xt{t}_{j}")
                nc.sync.dma_start(xt[:], xo_in[j, :, cs])
                ps_w = psA.tile([128, 512], F32, tag="mm", name=f"ps_w{t}_{j}")
                nc.tensor.matmul(ps_w[:], wwt[:, js], yt[:, cs])
                bn = work.tile([128, 512], F32, tag="bn", name=f"bn{t}_{j}")
                nc.scalar.activation(bn[:], ps_w[:], AF.Identity,
                                     bias=sh[j][:], scale=sc[j][:])
                ot = work.tile([128, 512], F32, tag="ot", name=f"ot{t}_{j}")
                nc.vector.tensor_add(ot[:], bn[:], xt[:])
                nc.sync.dma_start(out_d[j, :, cs], ot[:])

    nc.compile()
    return nc


def _make_runner(nc):
    """Cached jitted SPMD callable for one Bass module (mirrors
    bass2jax.run_bass_via_pjrt, but reusable across calls)."""
    import jax
    from jax.sharding import Mesh, PartitionSpec
    from jax.experimental.shard_map import shard_map

    bass2jax.install_neuronx_cc_hook()
    partition_name = (nc.partition_id_tensor.name
                      if nc.partition_id_tensor else None)
    in_names, out_names, out_avals, zero_shapes = [], [], [], []
    for alloc in nc.m.functions[0].allocations:
        if not isinstance(alloc, mybir.MemoryLocationSet):
            continue
        name = alloc.memorylocations[0].name
        if alloc.kind == "ExternalInput":
            if name != partition_name:
                in_names.append(name)
        elif alloc.kind == "ExternalOutput":
            shape = tuple(alloc.tensor_shape)
            dtype = mybir.dt.np(alloc.dtype)
            out_names.append(name)
            out_avals.append(jax.core.ShapedArray(shape, dtype))
            zero_shapes.append((shape, dtype))
    n_params = len(in_names)
    all_in_names = list(in_names) + list(out_names)
    if partition_name is not None:
        all_in_names.append(partition_name)
    donate = tuple(range(n_params, n_params + len(out_names)))

    def _body(*args):
        operands = list(args)
        if partition_name is not None:
            operands.append(bass2jax.partition_id_tensor())
        outs = bass2jax._bass_exec_p.bind(
            *operands,
            out_avals=tuple(out_avals),
            in_names=tuple(all_in_names),
            out_names=tuple(out_names),
            lowering_input_output_aliases=(),
            sim_require_finite=True,
            sim_require_nnan=True,
            nc=nc,
        )
        return tuple(outs)

    devices = jax.devices()[:NCORES]
    mesh = Mesh(np.asarray(devices), ("core",))
    in_specs = (PartitionSpec("core"),) * (n_params + len(out_names))
    out_specs = (PartitionSpec("core"),) * len(out_names)
    sharded = jax.jit(
        shard_map(_body, mesh=mesh, in_specs=in_specs, out_specs=out_specs,
                  check_rep=False),
        donate_argnums=donate, keep_unused=True)

    def run(in_maps):
        per_core = [[np.asarray(m[nm]) for nm in in_names] for m in in_maps]
        concat_in = [
            np.concatenate([per_core[c][i] for c in range(NCORES)], axis=0)
            for i in range(n_params)
        ]
        concat_zeros = [
            np.zeros((NCORES * sh_[0], *sh_[1:]), dt) for sh_, dt in zero_shapes
        ]
        out_arrs = sharded(*concat_in, *concat_zeros)
        return [
            {nm: np.asarray(out_arrs[i]).reshape(NCORES, *zero_shapes[i][0])[c]
             for i, nm in enumerate(out_names)}
            for c in range(NCORES)
        ]

    return run


def _runners():
    if "runA" not in _CACHE:
        _CACHE["runA"] = _make_runner(_build_nc_a())
        _CACHE["runB"] = _make_runner(_build_nc_b())
    return _CACHE["runA"], _CACHE["runB"]


def _prep(x, theta_w, theta_b, phi_w, phi_b, g_w, g_b, w_w, w_b, gamma, beta):
    xf = np.ascontiguousarray(x.reshape(B, C, N)).astype(np.float32)
    wcat = np.concatenate([phi_w.T, g_w.T], axis=1).astype(np.float32)
    bcat = np.concatenate([phi_b, g_b]).astype(np.float32).reshape(1, 2 * CI)
    thw = (theta_w.T / N).astype(np.float32)
    thb = (theta_b / N).astype(np.float32).reshape(CI, 1)
    wwt = np.ascontiguousarray(w_w.T).astype(np.float32)
    shared = {
        "wcat": wcat.reshape(2, 128, 2 * CI),
        "bcat": bcat,
        "thw": thw.reshape(2, 128, CI),
        "thb": thb,
        "wwt": wwt,
    }
    in_maps_a, xo_list = [], []
    for c in range(NCORES):
        b, h = divmod(c, 2)
        own = xf[b][:, h * HALF:(h + 1) * HALF]
        oth = xf[b][:, (1 - h) * HALF:(2 - h) * HALF]
        xp = np.concatenate([own, oth], axis=1).reshape(2, 128, N)
        in_maps_a.append({"x": np.ascontiguousarray(xp), **shared})
        xo_list.append(np.ascontiguousarray(own.reshape(2, 128, HALF)))
    return in_maps_a, xo_list, wwt


def kernel(**inputs):
    runA, runB = _runners()
    in_maps_a, xo_list, wwt = _prep(**inputs)
    res_a = runA(in_maps_a)

    # exact cross-core moment combination (equal counts of HALF per core)
    means = np.stack([res_a[c]["st"][:, :, 0].reshape(C) for c in range(NCORES)])
    varis = np.stack([res_a[c]["st"][:, :, 1].reshape(C) for c in range(NCORES)])
    m64, v64 = means.astype(np.float64), varis.astype(np.float64)
    mean_tot = m64.mean(axis=0)
    var_tot = v64.mean(axis=0) + (m64 ** 2).mean(axis=0) - mean_tot ** 2
    gamma = np.asarray(inputs["gamma"], np.float64)
    beta = np.asarray(inputs["beta"], np.float64)
    scale = gamma / np.sqrt(var_tot + EPS)
    shift = beta - mean_tot * scale
    sc = scale.astype(np.float32).reshape(2, 128, 1)
    sh = shift.astype(np.float32).reshape(2, 128, 1)

    in_maps_b = [
        {"yt": res_a[c]["yt"], "xo": xo_list[c], "wwt": wwt, "sc": sc, "sh": sh}
        for c in range(NCORES)
    ]
    res_b = runB(in_maps_b)

    out = np.empty((B, C, N), dtype=np.float32)
    for c in range(NCORES):
        b, h = divmod(c, 2)
        out[b][:, h * HALF:(h + 1) * HALF] = res_b[c]["out"].reshape(C, HALF)
    return out.reshape(B, C, H, W)
